# revision 2
# baseline (speedup 1.0000x reference)
"""Trainium2 Bass kernel for nn_EnhancedQuantumLayer (6-qubit circuit, B=32768).

Math: AngleEmbedding RX product state -> batch-independent 64x64 unitary
(StronglyEntanglingLayers + CNOT rings, precomputed on host from `weights`)
-> per-qubit PauliZ expectations.  Per sample: m = kron_q [cos a_q, sin a_q]
(64-vec), A = [Re;Im](C^T) m (128-vec), EV_q = sum_p sgn[p,q] A_p^2.

The backend's per-rep cost is dominated by per-instruction overhead with
partial cross-engine overlap, so the device program packs the work into 27
instructions/rep: 1 in-DMA (angles), 1 fat ACT Sin, 5 DVE kron muls, 1 DVE
StreamTranspose, 8 stage-1 matmuls (N=512 PSUM-bank limit), 1 fat ACT
Square over all 8 PSUM banks, 8 sign matmuls, 1 EV copy, 1 out-DMA.
Constants (packed C + signs) load once.  xt/scs/mtall are double-buffered;
semaphore thresholds use cumulative mixed-increment counts so every
instruction needs only its single allowed wait while rep i's front-end
(DMA/Sin/kron) overlaps rep i-1's back-end (matmuls/square/copy).
"""
import math
from contextlib import ExitStack

import numpy as np

import concourse.bass as bass
import concourse.mybir as mybir
from concourse.bass_utils import run_bass_kernel_spmd

F32 = mybir.dt.float32
NQ = 6
NL = 6
B = 32768
NCORES = 8
BC = B // NCORES          # 4096 samples per core


# ---------------------------------------------------------------- host precompute
def _host_matrices(weights):
    """(CcPacked (64,128) f32, SgnZ2 (128,6) f32) from weights (6,6,3)."""
    w = np.asarray(weights, dtype=np.float64)
    phi, theta, omega = w[..., 0], w[..., 1], w[..., 2]
    ct, st = np.cos(0.5 * theta), np.sin(0.5 * theta)
    em = np.exp(-0.5j * (phi + omega))
    ep = np.exp(0.5j * (phi + omega))
    epm = np.exp(0.5j * (phi - omega))
    emp = np.exp(-0.5j * (phi - omega))

    state = np.eye(64, dtype=np.complex128).reshape((64,) + (2,) * NQ)

    def apply_1q(state, U, q):
        ax = q + 1
        s = np.moveaxis(state, ax, -1)
        s = np.einsum('ij,...j->...i', U, s)
        return np.moveaxis(s, -1, ax)

    def cnot(state, c, t):
        ca, ta = c + 1, t + 1
        s0 = np.take(state, 0, axis=ca)
        s1 = np.take(state, 1, axis=ca)
        t_in = ta - 1 if ta > ca else ta
        s1 = np.flip(s1, axis=t_in)
        return np.stack([s0, s1], axis=ca)

    for l in range(NL):
        for q in range(NQ):
            U = np.array([
                [em[l, q] * ct[l, q], -epm[l, q] * st[l, q]],
                [emp[l, q] * st[l, q], ep[l, q] * ct[l, q]],
            ])
            state = apply_1q(state, U, q)
        r = (l % (NQ - 1)) + 1
        for q in range(NQ):
            state = cnot(state, q, (q + r) % NQ)

    stateF = state.reshape(64, 64)            # [in_e, out_o] = U[o, e]
    e = np.arange(64)
    pc = np.array([bin(v).count('1') for v in e])
    phase = (-1j) ** pc                       # (-i)^popcount: RX embedding phases
    Cc = phase[:, None] * stateF              # (64_in, 64_out)

    # device row j has qubit q at bit q; reference index e has qubit 0 as MSB
    bitrev = np.array([int(format(j, '06b')[::-1], 2) for j in range(64)])
    Cdev = Cc[bitrev, :]

    ccpacked = np.concatenate([Cdev.real, Cdev.imag], axis=1)   # (64, 128)

    o = np.arange(64)
    z = np.stack([1.0 - 2.0 * ((o >> (5 - q)) & 1) for q in range(NQ)], axis=1)
    sgn2 = np.concatenate([z, z], axis=0)                        # (128, 6)
    return ccpacked.astype(np.float32), sgn2.astype(np.float32)


def _lane_sample_index():
    """SL[L, sb]: sample_local for lane L, angle-block sb."""
    L = np.arange(128)
    h, jh, pl = L >> 6, (L >> 5) & 1, L & 31
    sb = np.arange(64)
    s, tp, p_hi = sb >> 4, (sb >> 2) & 3, sb & 3
    return (1024 * p_hi[None, :] + 32 * pl[:, None]
            + 8 * s[None, :] + 2 * tp[None, :] + h[:, None])


_SL = _lane_sample_index()


def _out_perm():
    """col j in device out (6, BC) holds sample_local perm[j].

    P col (within h group) c = 32*sb + pl with sb = 16*s + 4*tp + p_hi;
    sample_local = 1024*p_hi + 32*pl + 8*s + 2*tp + h.
    """
    perm = np.empty(BC, np.int64)
    c = np.arange(2048)
    pl = c % 32
    sbk = c // 32
    s = sbk // 16
    tp = (sbk // 4) % 4
    p_hi = sbk % 4
    base = 1024 * p_hi + 32 * pl + 8 * s + 2 * tp
    for h in range(2):
        perm[2048 * h + c] = base + h
    return perm


# ---------------------------------------------------------------- device program
def _build_bass(reps=1):
    nc = bass.Bass()
    xin = nc.dram_tensor("xin", [128, 966], F32, kind="ExternalInput")
    out = nc.dram_tensor("out", [NQ, BC], F32, kind="ExternalOutput")

    ctx = ExitStack()
    with ctx:
        sb = lambda nm, shape: ctx.enter_context(nc.sbuf_tensor(nm, shape, F32))
        sem = lambda nm: ctx.enter_context(nc.semaphore(name=nm))

        xt = [sb("xtA", [128, 832]), sb("xtB", [128, 832])]
        xc = sb("xc", [128, 134])
        scs = [sb("scsA", [128, 832]), sb("scsB", [128, 832])]
        k1b = sb("k1b", [128, 256])
        k2b = sb("k2b", [128, 256])
        k3b = sb("k3b", [128, 128])
        ub = sb("ub", [128, 512])
        mswz = sb("mswz", [128, 2048])
        mtall = [sb("mtallA", [128, 2048]), sb("mtallB", [128, 2048])]
        ppb = sb("ppb", [128, 4096])
        evs = sb("evs", [128, 4096])
        P = ctx.enter_context(nc.psum_tensor("P", [128, 4096], F32))

        Sd, Sa, Sv, Sp, Sq = (sem("Sd"), sem("Sa"), sem("Sv"), sem("Sp"),
                              sem("Sq"))

        cc2 = xc.ap()[:, 0:128]
        sg_t = xc.ap()[:, 128:134]

        def hsq(s_, q):
            return (s_.ap()[:, 0:768]
                    .rearrange("p (hf sb q) -> p sb hf q", hf=2, q=NQ)
                    [:, :, :, q:q + 1])

        block = ctx.enter_context(nc.Block())

        @block.sync
        def _(sync):
            dc = sync.dma_start(out=xc.ap()[:, :], in_=xin[:, 832:966])
            dc.then_inc(Sd, 16)
            for i in range(reps):
                d = sync.dma_start(out=xt[i % 2].ap()[:, :],
                                   in_=xin[:, 0:832])
                if i >= 2:
                    d._wait_ge(Sa, 2 * i - 3)      # sin_{i-2} read xt[i%2]
                d.then_inc(Sd, 16)
                o = sync.dma_start(out=out[:, :], in_=evs.ap()[0:NQ, :])
                o._wait_ge(Sv, 7 * (i + 1)).then_inc(Sq, 16)
            sync.wait_ge(Sq, 17 * reps)

        @block.scalar
        def _(scalar):
            sfn = mybir.ActivationFunctionType.Sin
            sqf = mybir.ActivationFunctionType.Square
            for i in range(reps):
                c_ = nc.scalar.activation(
                    scs[i % 2].ap()[:, :], xt[i % 2].ap()[:, :], sfn,
                )
                c_._wait_ge(Sd, 16 * (i + 2)).then_inc(Sa, 1)
                q1 = nc.scalar.activation(ppb.ap()[:, :], P.ap()[:, :], sqf)
                q1._wait_ge(Sp, 2 * (i + 1)).then_inc(Sa, 1)
                # Sq >= 17i+1: stage2_i (+1 x i+1) and outdma_{i-1} (+16 x i)
                e_ = nc.scalar.copy(evs.ap()[0:NQ, :], P.ap()[0:NQ, :])
                e_._wait_ge(Sq, 17 * i + 1).then_inc(Sv, 1)

        @block.vector
        def _(vector):
            for i in range(reps):
                sc = scs[i % 2]
                o1 = (k1b.ap()[:, :]
                      .rearrange("p (sb b1 b0) -> p sb b1 b0", b1=2, b0=2))
                i0 = hsq(sc, 0).squeeze(3).unsqueeze(2).broadcast_to((128, 64, 2, 2))
                i1 = hsq(sc, 1).squeeze(3).unsqueeze(3).broadcast_to((128, 64, 2, 2))
                t = nc.vector.tensor_mul(o1, i0, i1)
                t._wait_ge(Sa, 2 * i + 1).then_inc(Sv, 1)
                o2 = (k2b.ap()[:, :]
                      .rearrange("p (sb b3 b2) -> p sb b3 b2", b3=2, b2=2))
                i0 = hsq(sc, 2).squeeze(3).unsqueeze(2).broadcast_to((128, 64, 2, 2))
                i1 = hsq(sc, 3).squeeze(3).unsqueeze(3).broadcast_to((128, 64, 2, 2))
                t = nc.vector.tensor_mul(o2, i0, i1)
                t.then_inc(Sv, 1)
                o3 = (k3b.ap()[:, :]
                      .rearrange("p (sb b4) -> p sb b4", b4=2))
                i0 = hsq(sc, 4).squeeze(3)
                i1 = (sc.ap()[:, 768:832]
                      .rearrange("p (sb o) -> p sb o", o=1)
                      .broadcast_to((128, 64, 2)))
                t = nc.vector.tensor_mul(o3, i0, i1)
                t.then_inc(Sv, 1)
                # u = k3 (x) k2 : per block index 4*b4 + b32
                ou = (ub.ap()[:, :]
                      .rearrange("p (sb b4 b32) -> p sb b4 b32", b4=2, b32=4))
                i0 = (k3b.ap()[:, :].rearrange("p (sb b4) -> p sb b4", b4=2)
                      .unsqueeze(3).broadcast_to((128, 64, 2, 4)))
                i1 = (k2b.ap()[:, :].rearrange("p (sb b32) -> p sb b32", b32=4)
                      .unsqueeze(2).broadcast_to((128, 64, 2, 4)))
                t = nc.vector.tensor_mul(ou, i0, i1)
                t.then_inc(Sv, 1)
                # M = u (x) k1 : block col 16*b4 + 4*b32 + b10
                oM = (mswz.ap()[:, :]
                      .rearrange("p (sb u8 b10) -> p sb u8 b10", u8=8, b10=4))
                i0 = (ub.ap()[:, :].rearrange("p (sb u8) -> p sb u8", u8=8)
                      .unsqueeze(3).broadcast_to((128, 64, 8, 4)))
                i1 = (k1b.ap()[:, :].rearrange("p (sb b10) -> p sb b10", b10=4)
                      .unsqueeze(2).broadcast_to((128, 64, 8, 4)))
                t = nc.vector.tensor_mul(oM, i0, i1)
                t.then_inc(Sv, 1)
                st = nc.vector.transpose(mtall[i % 2].ap()[:, :],
                                         mswz.ap()[:, :])
                st.then_inc(Sv, 1)

        @block.tensor
        def _(tensor):
            for i in range(reps):
                mt = mtall[i % 2]
                for k in range(8):
                    h, s = divmod(k, 4)
                    mm = nc.tensor.matmul(
                        P.ap()[:, 2048 * h + 512 * s:2048 * h + 512 * (s + 1)],
                        cc2[64 * h:64 * h + 64, :],
                        mt.ap()[64 * h:64 * h + 64, 512 * s:512 * (s + 1)],
                        start=True, stop=True,
                    )
                    if k == 0:
                        # Sv >= 7i+6: transpose_i (6/rep) + evcopy_{i-1} (1/rep)
                        mm._wait_ge(Sv, 7 * i + 6)
                    if k == 7:
                        mm.then_inc(Sp, 2)
                for k in range(8):
                    h, s = divmod(k, 4)
                    mm = nc.tensor.matmul(
                        P.ap()[0:NQ, 2048 * h + 512 * s:2048 * h + 512 * (s + 1)],
                        sg_t,
                        ppb.ap()[:, 2048 * h + 512 * s:2048 * h + 512 * (s + 1)],
                        start=True, stop=True,
                    )
                    if k == 0:
                        mm._wait_ge(Sa, 2 * (i + 1))
                    if k == 7:
                        mm.then_inc(Sq, 1)

    return nc


_CACHE = {}


def _get_nc():
    if "nc" not in _CACHE:
        _CACHE["nc"] = _build_bass()
        _CACHE["perm"] = _out_perm()
    return _CACHE["nc"], _CACHE["perm"]


# ---------------------------------------------------------------- entry point
def _make_in_maps(x, weights, scale):
    x = np.asarray(x, dtype=np.float32)
    ccp, sg2 = _host_matrices(weights)
    hs = 0.5 * float(np.asarray(scale).reshape(-1)[0])
    a = x * hs                                   # (B, 6) half-angles
    L = np.arange(128)
    wbias = np.where(((L >> 5) & 1) == 0, math.pi / 2, 0.0).astype(np.float32)
    in_maps = []
    for k in range(NCORES):
        ak = a[k * BC:(k + 1) * BC]              # (4096, 6)
        lane = ak[_SL].reshape(128, 384)
        xs2 = np.empty((128, 966), np.float32)
        xs2[:, 0:384] = lane + np.float32(math.pi / 2)
        xs2[:, 384:768] = lane
        xs2[:, 768:832] = lane[:, 5::6] + wbias[:, None]
        xs2[0:64, 832:960] = ccp
        xs2[64:128, 832:960] = ccp
        xs2[:, 960:966] = sg2
        in_maps.append({"xin": xs2})
    return in_maps


def kernel(x, weights, scale):
    nc, perm = _get_nc()
    in_maps = _make_in_maps(x, weights, scale)
    res = run_bass_kernel_spmd(nc, in_maps, list(range(NCORES))).results
    ev = np.empty((B, NQ), np.float32)
    for k in range(NCORES):
        ev[k * BC + perm, :] = res[k]["out"].T
    return ev


if __name__ == "__main__":
    rng = np.random.default_rng(0)
    x = rng.standard_normal((B, NQ)).astype(np.float32)
    weights = rng.uniform(0, 2 * np.pi, (NL, NQ, 3)).astype(np.float32)
    scale = np.array([0.1], np.float32)
    ev = kernel(x, weights, scale)
    print("out", ev.shape, ev.dtype, ev[:2])
